# revision 19
# baseline (speedup 1.0000x reference)
"""ACN2d multi-branch attentive normalization on 8 TRN2 NeuronCores.

Sharding: data-parallel over batch B (8 samples -> 8 cores), no collectives.
Per core, a two-phase Bass/Tile kernel:
  phase 1 (point-major): x is cast to fp16 (DVE/ACT alternating) and
    transposed to point-major tiles by the XBAR DMA transpose (off the PE);
    att^T comes from per-tile PE matmuls; softmax over G=8 groups is a
    free-dim reduce on DVE; weighted moments P,Q accumulate on PE with a
    combined [x^T | x^2^T] moving operand (3D AP).
  phase 2 (channel-major): a^T transposes back to group-major chunks on PE
    (software-pipelined one chunk ahead of the A1/A2 matmuls); DVE/ACT/
    gpsimd combine out = x*A1 - A2 in fp16 and results stream to HBM as
    fp16 (host casts to fp32).
"""

import numpy as np

from concourse.bass_utils import run_bass_kernel_spmd


from contextlib import ExitStack

import concourse.bass as bass
import concourse.bacc as bacc
import concourse.tile as tile
from concourse import mybir
from concourse.masks import make_identity

F32 = mybir.dt.float32
F16 = mybir.dt.float16
AF = mybir.ActivationFunctionType
OP = mybir.AluOpType
AX = mybir.AxisListType

EPS = 1e-3
A_NORM_EPS = 1e-8


def bcast_last(ap, n):
    return bass.AP(tensor=ap.tensor, offset=ap.offset, ap=list(ap.ap) + [[0, n]])


def build_nc(N=32768):
    C, G = 128, 8
    TP = 128
    GRP = 2048
    JJ = GRP // TP                # tiles per group = 16
    ntiles = N // TP
    ngrp = N // GRP
    assert N % GRP == 0

    nc = bacc.Bacc("TRN2", target_bir_lowering=False, debug=False)
    x_ext = nc.declare_dram_parameter("x", [C, N], F32, isOutput=False).ap()
    w_ext = nc.declare_dram_parameter("conv_w", [G, C], F32, isOutput=False).ap()
    b_ext = nc.declare_dram_parameter("conv_b", [1, G], F32, isOutput=False).ap()
    out_ext = nc.declare_dram_parameter("out", [C, N], F16, isOutput=True).ap()

    with tile.TileContext(nc) as tc, ExitStack() as ctx:
        consts = ctx.enter_context(tc.tile_pool(name="consts", bufs=1))
        resident = ctx.enter_context(tc.tile_pool(name="resident", bufs=1))
        stats = ctx.enter_context(tc.tile_pool(name="stats", bufs=1))
        xstage = ctx.enter_context(tc.tile_pool(name="xstage", bufs=4))
        xqstage = ctx.enter_context(tc.tile_pool(name="xqstage", bufs=2))
        xt2stage = ctx.enter_context(tc.tile_pool(name="xt2stage", bufs=3))
        estage = ctx.enter_context(tc.tile_pool(name="estage", bufs=2))
        zstage = ctx.enter_context(tc.tile_pool(name="zstage", bufs=2))
        ag_sbp = ctx.enter_context(tc.tile_pool(name="ag_sb", bufs=3))
        a1s_pool = ctx.enter_context(tc.tile_pool(name="a1s_sb", bufs=3))
        tm_pool = ctx.enter_context(tc.tile_pool(name="tm_sb", bufs=3))
        opool = ctx.enter_context(tc.tile_pool(name="ostage", bufs=2))

        # ---- constants ----
        ident = consts.tile([128, 128], F16)
        make_identity(nc, ident)
        ident8 = consts.tile([G, G], F32)
        make_identity(nc, ident8)
        onesf32 = consts.tile([128, 1], F32)
        nc.vector.memset(onesf32, 1.0)
        eps_t = consts.tile([G, 1], F32)
        nc.vector.memset(eps_t, EPS)
        w_sb = consts.tile([G, C], F32)
        nc.sync.dma_start(w_sb[:], w_ext)
        b_row = consts.tile([1, G], F32)
        nc.sync.dma_start(b_row[:], b_ext)
        ones_col = consts.tile([1, 128], F16)
        nc.gpsimd.memset(ones_col, 1.0)
        b_rep = consts.tile([1, JJ, G], F16)
        nc.gpsimd.tensor_copy(
            b_rep[:],
            bass.AP(tensor=b_row[:].tensor, offset=b_row[:].offset,
                    ap=[b_row[:].ap[0], [0, JJ], b_row[:].ap[1]]))
        w_g = consts.tile([G, C], F32)
        nc.gpsimd.tensor_copy(w_g[:], w_sb[:])

        with tc.tile_pool(name="ph0psum", bufs=1, space="PSUM") as ph0psum:
            wT_ps = ph0psum.tile([C, G], F32)
            nc.tensor.transpose(wT_ps[:], w_g[:], ident8[:])
            wT = consts.tile([C, G], F16)
            nc.scalar.copy(wT[:], wT_ps[:])

        # ---- residents ----
        xc = resident.tile([C, N], F16)
        a_t = resident.tile([128, N // 16], F16)

        # ================= phase 1 =================
        with tc.tile_pool(name="att_ps", bufs=2, space="PSUM") as att_pool, \
             tc.tile_pool(name="pq_ps", bufs=1, space="PSUM") as pq_pool:
            pq = pq_pool.tile([G, 2 * C], F32)   # [P^T | Q^T] interleaved

            def emit_pq(gg, xt2p):
                for j in range(JJ):
                    t = gg * JJ + j
                    nc.tensor.matmul(
                        pq[:].rearrange("p (s q) -> p s q", s=2),
                        lhsT=a_t[:, 8 * t:8 * (t + 1)],
                        rhs=bass.AP(
                            tensor=xt2p[:].tensor, offset=xt2p[:].offset
                            + j * TP * xt2p[:].ap[-1][0],
                            ap=[xt2p[:].ap[0], xt2p[:].ap[1],
                                [xt2p[:].ap[-1][0], TP]]),
                        start=(t == 0), stop=(t == ntiles - 1),
                        skip_group_check=True)

            # x loads stream on the ACT hwdge queue, prefetched 2 groups
            # ahead; the XBAR transposes ride the SP queue so neither DMA
            # stream queues behind the other's semaphore waits.
            xs_tiles = {}
            for gp in range(min(2, ngrp)):
                xs_tiles[gp] = xstage.tile([C, GRP], F32, name="xs")
                nc.scalar.dma_start(xs_tiles[gp][:],
                                    x_ext[:, gp * GRP:(gp + 1) * GRP])

            prev = None  # previous group's xt2 tile for deferred P/Q
            for g in range(ngrp):
                n0 = g * GRP
                if g + 2 < ngrp:
                    xs_tiles[g + 2] = xstage.tile([C, GRP], F32,
                                                  name="xs")
                    nc.scalar.dma_start(
                        xs_tiles[g + 2][:],
                        x_ext[:, (g + 2) * GRP:(g + 3) * GRP])
                xs = xs_tiles.pop(g)
                xcs = xc[:, n0:n0 + GRP]
                nc.vector.tensor_copy(xcs, xs[:])       # cast f32 -> f16

                # XBAR DMA transpose (SP queue): xt2[:, 0, :] = x^T tiles;
                # x^2 follows on ACT as a contiguous 2D Square of plane 0
                xt2 = xt2stage.tile([128, 2, GRP], F16)
                nc.sync.dma_start_transpose(
                    xt2[:, 0, :].rearrange("p (j q) -> p j q", q=TP), xcs)
                nc.scalar.activation(xt2[:, 1, :], xt2[:, 0, :], AF.Square)

                # deferred P/Q first: it is dep-ready, so the PE never idles
                # waiting for this group's cast
                if prev is not None:
                    emit_pq(*prev)

                att = att_pool.tile([128, JJ, G], F32)
                for j in range(JJ):
                    xcj = xc[:, n0 + j * TP:n0 + (j + 1) * TP]
                    nc.tensor.matmul(
                        att[:, j, :], lhsT=xcj, rhs=wT[:],
                        start=(j == 0), stop=False, skip_group_check=True)
                nc.tensor.matmul(
                    att[:].rearrange("p j g -> p (j g)"),
                    lhsT=ones_col[:], rhs=b_rep[:].rearrange("p j g -> p (j g)"),
                    start=False, stop=True, skip_group_check=True)

                e = estage.tile([128, JJ, G], F32)
                nc.scalar.activation(e[:], att[:], AF.Exp)
                z = zstage.tile([128, JJ], F32)
                nc.vector.tensor_reduce(z[:], e[:], axis=AX.X, op=OP.add)
                rz = zstage.tile([128, JJ], F32)
                nc.vector.reciprocal(rz[:], z[:])
                atG = a_t[:, g * (GRP // 16):(g + 1) * (GRP // 16)].rearrange(
                    "p (j g) -> p j g", g=G)
                nc.vector.tensor_tensor(atG, e[:], bcast_last(rz[:], G), op=OP.mult)
                prev = (g, xt2)

            emit_pq(*prev)

            # ================= phase 1.5: statistics =================
            partials = stats.tile([128, G], F32)
            nc.vector.tensor_reduce(
                partials[:], a_t[:].rearrange("p (t g) -> p g t", g=G),
                axis=AX.X, op=OP.add)
            with tc.tile_pool(name="s_ps", bufs=1, space="PSUM") as s_pool:
                s_ps = s_pool.tile([G, 1], F32)
                nc.tensor.matmul(s_ps[:], lhsT=partials[:], rhs=onesf32[:],
                                 start=True, stop=True)
                s_eps = stats.tile([G, 1], F32)
                nc.vector.tensor_scalar_add(s_eps[:], s_ps[:], A_NORM_EPS)
                sden = stats.tile([G, 1], F32)
                nc.vector.reciprocal(sden[:], s_eps[:])
                T = stats.tile([G, 1], F32)
                nc.vector.tensor_tensor(T[:], s_ps[:], sden[:], op=OP.mult)
                meanT = stats.tile([G, C], F32)
                nc.vector.tensor_scalar_mul(meanT[:], pq[:, 0:C], sden[:])
                m2T = stats.tile([G, C], F32)
                nc.vector.tensor_scalar_mul(m2T[:], pq[:, C:2 * C], sden[:])
            u = stats.tile([G, 1], F32)
            nc.vector.tensor_scalar(u[:], T[:], -1.0, 2.0, op0=OP.mult, op1=OP.add)
            meansq = stats.tile([G, C], F32)
            nc.vector.tensor_tensor(meansq[:], meanT[:], meanT[:], op=OP.mult)
            tmpv = stats.tile([G, C], F32)
            nc.vector.tensor_scalar_mul(tmpv[:], meansq[:], u[:])
            varT = stats.tile([G, C], F32)
            nc.vector.tensor_tensor(varT[:], m2T[:], tmpv[:], op=OP.subtract)
            # inv_std = exp(-0.5*ln(var+eps)): Ln+Exp live in one ACT table
            # set, so no table switch on the phase-1.5 critical path
            lnv = stats.tile([G, C], F32)
            nc.scalar.activation(lnv[:], varT[:], AF.Ln, bias=eps_t[:])
            invT = stats.tile([G, C], F32)
            nc.scalar.activation(invT[:], lnv[:], AF.Exp, scale=-0.5)
            Ff = stats.tile([G, C], F32)
            nc.vector.tensor_tensor(Ff[:], meanT[:], invT[:], op=OP.mult)
            E = stats.tile([G, C], F16)
            nc.scalar.copy(E[:], invT[:])
            F = stats.tile([G, C], F16)
            nc.scalar.copy(F[:], Ff[:])

        # ================= phase 2: apply =================
        # pipelined: chunk cc's group-major transposes run on PE while chunk
        # cc-1 flows through eviction -> A1/A2 matmuls -> elementwise -> DMA
        NCH = 1024
        HC = 512
        nchunk = N // NCH
        with tc.tile_pool(name="agp_ps", bufs=3, space="PSUM") as agp_pool, \
             tc.tile_pool(name="a1_ps", bufs=2, space="PSUM") as a1_psum, \
             tc.tile_pool(name="a2_ps", bufs=2, space="PSUM") as a2_psum:

            def emit_transposes(cc):
                agp = agp_pool.tile([G, NCH], F16)
                for r in range(NCH // TP):
                    t = cc * (NCH // TP) + r
                    nc.tensor.matmul(
                        agp[:, r * TP:(r + 1) * TP],
                        lhsT=a_t[:, 8 * t:8 * (t + 1)], rhs=ident[:],
                        is_transpose=True, start=(r == 0),
                        stop=(r == NCH // TP - 1), skip_group_check=True)
                ags = ag_sbp.tile([G, NCH], F16)
                nc.vector.tensor_copy(ags[:, 0:HC], agp[:, 0:HC])
                nc.scalar.copy(ags[:, HC:NCH], agp[:, HC:NCH])
                return ags

            def emit_apply(cc, ags):
                n0 = cc * NCH
                os = opool.tile([C, NCH], F16)
                for h in range(2):
                    m0 = h * HC
                    a1p = a1_psum.tile([C, HC], F32)
                    a2p = a2_psum.tile([C, HC], F32)
                    nc.tensor.matmul(a1p[:], lhsT=E[:], rhs=ags[:, m0:m0 + HC],
                                     start=True, stop=True, skip_group_check=True)
                    nc.tensor.matmul(a2p[:], lhsT=F[:], rhs=ags[:, m0:m0 + HC],
                                     start=True, stop=True, skip_group_check=True)
                    a1s = a1s_pool.tile([C, HC], F16)
                    nc.scalar.copy(a1s[:], a1p[:])
                    tm = tm_pool.tile([C, HC], F16)
                    if (2 * cc + h) % 2 == 0:
                        nc.gpsimd.tensor_tensor(
                            tm[:], xc[:, n0 + m0:n0 + m0 + HC], a1s[:],
                            op=OP.mult)
                    else:
                        nc.vector.tensor_tensor(
                            tm[:], xc[:, n0 + m0:n0 + m0 + HC], a1s[:],
                            op=OP.mult)
                    nc.vector.tensor_tensor(os[:, m0:m0 + HC], tm[:], a2p[:],
                                            op=OP.subtract)
                nc.sync.dma_start(out_ext[:, n0:n0 + NCH], os[:])

            # deferral depth 2: the PE always has two chunks of transpose
            # work queued ahead of the eviction-dependent A1/A2 matmuls
            ags_hist = []
            for cc in range(nchunk):
                ags_hist.append(emit_transposes(cc))
                if cc >= 2:
                    emit_apply(cc - 2, ags_hist[cc - 2])
            emit_apply(nchunk - 2, ags_hist[nchunk - 2])
            emit_apply(nchunk - 1, ags_hist[nchunk - 1])

    nc.compile()
    return nc


_CACHED_NC = None


def kernel(x, conv_w, conv_b):
    global _CACHED_NC
    x = np.asarray(x)
    conv_w = np.ascontiguousarray(conv_w, dtype=np.float32)
    conv_b = np.asarray(conv_b, dtype=np.float32)
    b, c, n = x.shape[0], x.shape[1], x.shape[2]
    if _CACHED_NC is None:
        _CACHED_NC = build_nc(N=n)
    nc = _CACHED_NC

    in_maps = [
        {
            "x": np.ascontiguousarray(x[i, :, :, 0], dtype=np.float32),
            "conv_w": conv_w,
            "conv_b": conv_b.reshape(1, -1),
        }
        for i in range(b)
    ]
    res = run_bass_kernel_spmd(nc, in_maps, core_ids=list(range(b)))
    out = np.stack([res.results[i]["out"] for i in range(b)])[..., None]
    return out.astype(np.float32)


# revision 21
# speedup vs baseline: 1.0471x; 1.0471x over previous
"""ACN2d multi-branch attentive normalization on 8 TRN2 NeuronCores.

Sharding: data-parallel over batch B (8 samples -> 8 cores), no collectives.
Per core, a two-phase Bass/Tile kernel:
  phase 1 (point-major): x is cast to fp16 (DVE/ACT alternating) and
    transposed to point-major tiles by the XBAR DMA transpose (off the PE);
    att^T comes from per-tile PE matmuls; softmax over G=8 groups is a
    free-dim reduce on DVE; weighted moments P,Q accumulate on PE with a
    combined [x^T | x^2^T] moving operand (3D AP).
  phase 2 (channel-major): a^T transposes back to group-major chunks on PE
    (software-pipelined one chunk ahead of the A1/A2 matmuls); DVE/ACT/
    gpsimd combine out = x*A1 - A2 in fp16 and results stream to HBM as
    fp16 (host casts to fp32).
"""

import numpy as np

from concourse.bass_utils import run_bass_kernel_spmd


from contextlib import ExitStack

import concourse.bass as bass
import concourse.bacc as bacc
import concourse.tile as tile
from concourse import mybir
from concourse.masks import make_identity

F32 = mybir.dt.float32
F16 = mybir.dt.float16
AF = mybir.ActivationFunctionType
OP = mybir.AluOpType
AX = mybir.AxisListType

EPS = 1e-3
A_NORM_EPS = 1e-8


def bcast_last(ap, n):
    return bass.AP(tensor=ap.tensor, offset=ap.offset, ap=list(ap.ap) + [[0, n]])


def build_nc(N=32768):
    C, G = 128, 8
    TP = 128
    GRP = 2048
    JJ = GRP // TP                # tiles per group = 16
    ntiles = N // TP
    ngrp = N // GRP
    assert N % GRP == 0

    nc = bacc.Bacc("TRN2", target_bir_lowering=False, debug=False)
    x_ext = nc.declare_dram_parameter("x", [C, N], F32, isOutput=False).ap()
    w_ext = nc.declare_dram_parameter("conv_w", [G, C], F32, isOutput=False).ap()
    b_ext = nc.declare_dram_parameter("conv_b", [1, G], F32, isOutput=False).ap()
    out_ext = nc.declare_dram_parameter("out", [C, N], F16, isOutput=True).ap()

    with tile.TileContext(nc) as tc, ExitStack() as ctx:
        consts = ctx.enter_context(tc.tile_pool(name="consts", bufs=1))
        resident = ctx.enter_context(tc.tile_pool(name="resident", bufs=1))
        stats = ctx.enter_context(tc.tile_pool(name="stats", bufs=1))
        xstage = ctx.enter_context(tc.tile_pool(name="xstage", bufs=4))
        xqstage = ctx.enter_context(tc.tile_pool(name="xqstage", bufs=2))
        xt2stage = ctx.enter_context(tc.tile_pool(name="xt2stage", bufs=3))
        estage = ctx.enter_context(tc.tile_pool(name="estage", bufs=2))
        zstage = ctx.enter_context(tc.tile_pool(name="zstage", bufs=2))
        ag_sbp = ctx.enter_context(tc.tile_pool(name="ag_sb", bufs=3))
        a1s_pool = ctx.enter_context(tc.tile_pool(name="a1s_sb", bufs=3))
        tm_pool = ctx.enter_context(tc.tile_pool(name="tm_sb", bufs=3))
        opool = ctx.enter_context(tc.tile_pool(name="ostage", bufs=2))

        # ---- constants ----
        ident = consts.tile([128, 128], F16)
        make_identity(nc, ident)
        ident8 = consts.tile([G, G], F32)
        make_identity(nc, ident8)
        onesf32 = consts.tile([128, 1], F32)
        nc.vector.memset(onesf32, 1.0)
        eps_t = consts.tile([G, 1], F32)
        nc.vector.memset(eps_t, EPS)
        w_sb = consts.tile([G, C], F32)
        nc.sync.dma_start(w_sb[:], w_ext)
        b_row = consts.tile([1, G], F32)
        nc.sync.dma_start(b_row[:], b_ext)
        ones_col = consts.tile([1, 128], F16)
        nc.gpsimd.memset(ones_col, 1.0)
        b_rep = consts.tile([1, JJ, G], F16)
        nc.gpsimd.tensor_copy(
            b_rep[:],
            bass.AP(tensor=b_row[:].tensor, offset=b_row[:].offset,
                    ap=[b_row[:].ap[0], [0, JJ], b_row[:].ap[1]]))
        w_g = consts.tile([G, C], F32)
        nc.gpsimd.tensor_copy(w_g[:], w_sb[:])

        with tc.tile_pool(name="ph0psum", bufs=1, space="PSUM") as ph0psum:
            wT_ps = ph0psum.tile([C, G], F32)
            nc.tensor.transpose(wT_ps[:], w_g[:], ident8[:])
            wT = consts.tile([C, G], F16)
            nc.scalar.copy(wT[:], wT_ps[:])

        # ---- residents ----
        xc = resident.tile([C, N], F16)
        a_t = resident.tile([128, N // 16], F16)

        # ================= phase 1 =================
        with tc.tile_pool(name="att_ps", bufs=2, space="PSUM") as att_pool, \
             tc.tile_pool(name="pq_ps", bufs=1, space="PSUM") as pq_pool:
            pq = pq_pool.tile([G, 2 * C], F32)   # [P^T | Q^T] interleaved

            def emit_pq(gg, xt2p):
                for j in range(JJ):
                    t = gg * JJ + j
                    nc.tensor.matmul(
                        pq[:].rearrange("p (s q) -> p s q", s=2),
                        lhsT=a_t[:, 8 * t:8 * (t + 1)],
                        rhs=bass.AP(
                            tensor=xt2p[:].tensor, offset=xt2p[:].offset
                            + j * TP * xt2p[:].ap[-1][0],
                            ap=[xt2p[:].ap[0], xt2p[:].ap[1],
                                [xt2p[:].ap[-1][0], TP]]),
                        start=(t == 0), stop=(t == ntiles - 1),
                        skip_group_check=True)

            # x loads stream on the ACT hwdge queue, prefetched 2 groups
            # ahead; the XBAR transposes ride the SP queue so neither DMA
            # stream queues behind the other's semaphore waits.
            xs_tiles = {}
            for gp in range(min(2, ngrp)):
                xs_tiles[gp] = xstage.tile([C, GRP], F32, name="xs")
                nc.scalar.dma_start(xs_tiles[gp][:],
                                    x_ext[:, gp * GRP:(gp + 1) * GRP])

            prevs = []  # deferred groups' xt2 tiles for P/Q
            for g in range(ngrp):
                n0 = g * GRP
                if g + 2 < ngrp:
                    xs_tiles[g + 2] = xstage.tile([C, GRP], F32,
                                                  name="xs")
                    nc.scalar.dma_start(
                        xs_tiles[g + 2][:],
                        x_ext[:, (g + 2) * GRP:(g + 3) * GRP])
                xs = xs_tiles.pop(g)
                xcs = xc[:, n0:n0 + GRP]
                nc.vector.tensor_copy(xcs, xs[:])       # cast f32 -> f16

                # XBAR DMA transpose (SP queue): xt2[:, 0, :] = x^T tiles
                xt2 = xt2stage.tile([128, 2, GRP], F16)
                nc.sync.dma_start_transpose(
                    xt2[:, 0, :].rearrange("p (j q) -> p j q", q=TP), xcs)

                # deferred P/Q first (two groups back): dep-ready, so the PE
                # never idles waiting on this group's cast->xbar->x^2 chain
                if len(prevs) >= 2:
                    emit_pq(*prevs.pop(0))

                att = att_pool.tile([128, JJ, G], F32)
                for j in range(JJ):
                    xcj = xc[:, n0 + j * TP:n0 + (j + 1) * TP]
                    nc.tensor.matmul(
                        att[:, j, :], lhsT=xcj, rhs=wT[:],
                        start=(j == 0), stop=False, skip_group_check=True)
                nc.tensor.matmul(
                    att[:].rearrange("p j g -> p (j g)"),
                    lhsT=ones_col[:], rhs=b_rep[:].rearrange("p j g -> p (j g)"),
                    start=False, stop=True, skip_group_check=True)

                e = estage.tile([128, JJ, G], F32)
                nc.scalar.activation(e[:], att[:], AF.Exp)
                z = zstage.tile([128, JJ], F32)
                nc.vector.tensor_reduce(z[:], e[:], axis=AX.X, op=OP.add)
                rz = zstage.tile([128, JJ], F32)
                nc.vector.reciprocal(rz[:], z[:])
                atG = a_t[:, g * (GRP // 16):(g + 1) * (GRP // 16)].rearrange(
                    "p (j g) -> p j g", g=G)
                nc.vector.tensor_tensor(atG, e[:], bcast_last(rz[:], G), op=OP.mult)

                # x^2 from the transposed plane, split across DVE and ACT so
                # both halves land in ~1us right after the XBAR completes
                nc.vector.tensor_tensor(
                    xt2[:, 1, 0:GRP // 2], xt2[:, 0, 0:GRP // 2],
                    xt2[:, 0, 0:GRP // 2], op=OP.mult)
                nc.scalar.activation(
                    xt2[:, 1, GRP // 2:GRP], xt2[:, 0, GRP // 2:GRP], AF.Square)
                prevs.append((g, xt2))

            for pv in prevs:
                emit_pq(*pv)

            # ================= phase 1.5: statistics =================
            partials = stats.tile([128, G], F32)
            nc.vector.tensor_reduce(
                partials[:], a_t[:].rearrange("p (t g) -> p g t", g=G),
                axis=AX.X, op=OP.add)
            with tc.tile_pool(name="s_ps", bufs=1, space="PSUM") as s_pool:
                s_ps = s_pool.tile([G, 1], F32)
                nc.tensor.matmul(s_ps[:], lhsT=partials[:], rhs=onesf32[:],
                                 start=True, stop=True)
                s_eps = stats.tile([G, 1], F32)
                nc.vector.tensor_scalar_add(s_eps[:], s_ps[:], A_NORM_EPS)
                sden = stats.tile([G, 1], F32)
                nc.vector.reciprocal(sden[:], s_eps[:])
                T = stats.tile([G, 1], F32)
                nc.vector.tensor_tensor(T[:], s_ps[:], sden[:], op=OP.mult)
                meanT = stats.tile([G, C], F32)
                nc.vector.tensor_scalar_mul(meanT[:], pq[:, 0:C], sden[:])
                m2T = stats.tile([G, C], F32)
                nc.vector.tensor_scalar_mul(m2T[:], pq[:, C:2 * C], sden[:])
            u = stats.tile([G, 1], F32)
            nc.vector.tensor_scalar(u[:], T[:], -1.0, 2.0, op0=OP.mult, op1=OP.add)
            meansq = stats.tile([G, C], F32)
            nc.vector.tensor_tensor(meansq[:], meanT[:], meanT[:], op=OP.mult)
            tmpv = stats.tile([G, C], F32)
            nc.vector.tensor_scalar_mul(tmpv[:], meansq[:], u[:])
            varT = stats.tile([G, C], F32)
            nc.vector.tensor_tensor(varT[:], m2T[:], tmpv[:], op=OP.subtract)
            # inv_std = exp(-0.5*ln(var+eps)): Ln+Exp live in one ACT table
            # set, so no table switch on the phase-1.5 critical path
            lnv = stats.tile([G, C], F32)
            nc.scalar.activation(lnv[:], varT[:], AF.Ln, bias=eps_t[:])
            invT = stats.tile([G, C], F32)
            nc.scalar.activation(invT[:], lnv[:], AF.Exp, scale=-0.5)
            Ff = stats.tile([G, C], F32)
            nc.vector.tensor_tensor(Ff[:], meanT[:], invT[:], op=OP.mult)
            E = stats.tile([G, C], F16)
            nc.scalar.copy(E[:], invT[:])
            F = stats.tile([G, C], F16)
            nc.scalar.copy(F[:], Ff[:])

        # ================= phase 2: apply =================
        # pipelined: chunk cc's group-major transposes run on PE while chunk
        # cc-1 flows through eviction -> A1/A2 matmuls -> elementwise -> DMA
        NCH = 1024
        HC = 512
        nchunk = N // NCH
        with tc.tile_pool(name="agp_ps", bufs=3, space="PSUM") as agp_pool, \
             tc.tile_pool(name="a1_ps", bufs=2, space="PSUM") as a1_psum, \
             tc.tile_pool(name="a2_ps", bufs=2, space="PSUM") as a2_psum:

            def emit_transposes(cc):
                agp = agp_pool.tile([G, NCH], F16)
                for r in range(NCH // TP):
                    t = cc * (NCH // TP) + r
                    nc.tensor.matmul(
                        agp[:, r * TP:(r + 1) * TP],
                        lhsT=a_t[:, 8 * t:8 * (t + 1)], rhs=ident[:],
                        is_transpose=True, start=(r == 0),
                        stop=(r == NCH // TP - 1), skip_group_check=True)
                ags = ag_sbp.tile([G, NCH], F16)
                nc.vector.tensor_copy(ags[:, 0:HC], agp[:, 0:HC])
                nc.scalar.copy(ags[:, HC:NCH], agp[:, HC:NCH])
                return ags

            def emit_apply(cc, ags):
                n0 = cc * NCH
                os = opool.tile([C, NCH], F16)
                for h in range(2):
                    m0 = h * HC
                    a1p = a1_psum.tile([C, HC], F32)
                    a2p = a2_psum.tile([C, HC], F32)
                    nc.tensor.matmul(a1p[:], lhsT=E[:], rhs=ags[:, m0:m0 + HC],
                                     start=True, stop=True, skip_group_check=True)
                    nc.tensor.matmul(a2p[:], lhsT=F[:], rhs=ags[:, m0:m0 + HC],
                                     start=True, stop=True, skip_group_check=True)
                    a1s = a1s_pool.tile([C, HC], F16)
                    nc.scalar.copy(a1s[:], a1p[:])
                    tm = tm_pool.tile([C, HC], F16)
                    if (2 * cc + h) % 2 == 0:
                        nc.gpsimd.tensor_tensor(
                            tm[:], xc[:, n0 + m0:n0 + m0 + HC], a1s[:],
                            op=OP.mult)
                    else:
                        nc.vector.tensor_tensor(
                            tm[:], xc[:, n0 + m0:n0 + m0 + HC], a1s[:],
                            op=OP.mult)
                    nc.vector.tensor_tensor(os[:, m0:m0 + HC], tm[:], a2p[:],
                                            op=OP.subtract)
                nc.sync.dma_start(out_ext[:, n0:n0 + NCH], os[:])

            # deferral depth 2: the PE always has two chunks of transpose
            # work queued ahead of the eviction-dependent A1/A2 matmuls
            ags_hist = []
            for cc in range(nchunk):
                ags_hist.append(emit_transposes(cc))
                if cc >= 2:
                    emit_apply(cc - 2, ags_hist[cc - 2])
            emit_apply(nchunk - 2, ags_hist[nchunk - 2])
            emit_apply(nchunk - 1, ags_hist[nchunk - 1])

    nc.compile()
    return nc


_CACHED_NC = None


def kernel(x, conv_w, conv_b):
    global _CACHED_NC
    x = np.asarray(x)
    conv_w = np.ascontiguousarray(conv_w, dtype=np.float32)
    conv_b = np.asarray(conv_b, dtype=np.float32)
    b, c, n = x.shape[0], x.shape[1], x.shape[2]
    if _CACHED_NC is None:
        _CACHED_NC = build_nc(N=n)
    nc = _CACHED_NC

    in_maps = [
        {
            "x": np.ascontiguousarray(x[i, :, :, 0], dtype=np.float32),
            "conv_w": conv_w,
            "conv_b": conv_b.reshape(1, -1),
        }
        for i in range(b)
    ]
    res = run_bass_kernel_spmd(nc, in_maps, core_ids=list(range(b)))
    out = np.stack([res.results[i]["out"] for i in range(b)])[..., None]
    return out.astype(np.float32)


# revision 23
# speedup vs baseline: 1.2296x; 1.1744x over previous
"""ACN2d multi-branch attentive normalization on 8 TRN2 NeuronCores.

Sharding: data-parallel over batch B (8 samples -> 8 cores), no collectives.
Per core, a two-phase Bass/Tile kernel:
  phase 1 (point-major): PE transposes x to fp16 point-major tiles and
    computes att^T; softmax over the G=8 groups is a free-dim reduce on DVE;
    weighted moments P = x a^T, Q = x^2 a^T and s = sum_n a accumulate on PE.
  phase 2 (channel-major): a^T transposes back to group-major chunks on PE;
    A1/A2 are K=8 matmuls against inv_std^T / (mean inv_std)^T; gpsimd/DVE
    combine out = x*A1 - A2 and results stream to HBM.
"""

import numpy as np

from concourse.bass_utils import run_bass_kernel_spmd


from contextlib import ExitStack

import concourse.bass as bass
import concourse.bacc as bacc
import concourse.tile as tile
from concourse import mybir
from concourse.masks import make_identity

F32 = mybir.dt.float32
F16 = mybir.dt.float16
AF = mybir.ActivationFunctionType
OP = mybir.AluOpType
AX = mybir.AxisListType

EPS = 1e-3
A_NORM_EPS = 1e-8


def bcast_last(ap, n):
    return bass.AP(tensor=ap.tensor, offset=ap.offset, ap=list(ap.ap) + [[0, n]])


def build_nc(N=32768, fold_bias=True):
    C, G = 128, 8
    TP = 128
    GRP = 2048
    JJ = GRP // TP                # tiles per group = 16
    ntiles = N // TP
    ngrp = N // GRP
    assert N % GRP == 0

    nc = bacc.Bacc("TRN2", target_bir_lowering=False, debug=False)
    x_ext = nc.declare_dram_parameter("x", [C, N], F32, isOutput=False).ap()
    w_ext = nc.declare_dram_parameter("conv_w", [G, C], F32, isOutput=False).ap()
    b_ext = nc.declare_dram_parameter("conv_b", [1, G], F32, isOutput=False).ap()
    out_ext = nc.declare_dram_parameter("out", [C, N], F16, isOutput=True).ap()

    with tile.TileContext(nc) as tc, ExitStack() as ctx:
        consts = ctx.enter_context(tc.tile_pool(name="consts", bufs=1))
        resident = ctx.enter_context(tc.tile_pool(name="resident", bufs=1))
        stats = ctx.enter_context(tc.tile_pool(name="stats", bufs=1))
        # SBUF staging pools stay open for the whole kernel so later pools
        # never inherit WAW deps on the DMA lanes that filled earlier ones.
        xstage = ctx.enter_context(tc.tile_pool(name="xstage", bufs=2))
        x2stage = ctx.enter_context(tc.tile_pool(name="x2stage", bufs=2))
        estage = ctx.enter_context(tc.tile_pool(name="estage", bufs=2))
        zstage = ctx.enter_context(tc.tile_pool(name="zstage", bufs=2))
        ag_sbp = ctx.enter_context(tc.tile_pool(name="ag_sb", bufs=2))
        a12_sbp = ctx.enter_context(tc.tile_pool(name="a12_sb", bufs=2))
        tmp_pool = ctx.enter_context(tc.tile_pool(name="tmp_sb", bufs=2))
        opool = ctx.enter_context(tc.tile_pool(name="ostage", bufs=2))

        # ---- constants: everything the PE touches funnels through gpsimd ----
        ident = consts.tile([128, 128], F16)
        make_identity(nc, ident)
        ident8 = consts.tile([G, G], F32)
        make_identity(nc, ident8)
        onesf32 = consts.tile([128, 1], F32)
        nc.vector.memset(onesf32, 1.0)
        eps_t = consts.tile([G, 1], F32)
        nc.vector.memset(eps_t, EPS)
        w_sb = consts.tile([G, C], F32)
        nc.sync.dma_start(w_sb[:], w_ext)
        b_row = consts.tile([1, G], F32)
        nc.sync.dma_start(b_row[:], b_ext)
        ones_col = consts.tile([1, 128], F16)
        nc.gpsimd.memset(ones_col, 1.0)
        b_rep = consts.tile([1, JJ, G], F16)
        nc.gpsimd.tensor_copy(
            b_rep[:],
            bass.AP(tensor=b_row[:].tensor, offset=b_row[:].offset,
                    ap=[b_row[:].ap[0], [0, JJ], b_row[:].ap[1]]))
        # bounce conv_w through gpsimd so the wT transpose waits on Pool only
        w_g = consts.tile([G, C], F32)
        nc.gpsimd.tensor_copy(w_g[:], w_sb[:])

        with tc.tile_pool(name="ph0psum", bufs=1, space="PSUM") as ph0psum:
            wT_ps = ph0psum.tile([C, G], F32)
            nc.tensor.transpose(wT_ps[:], w_g[:], ident8[:])
            wT = consts.tile([C, G], F16)
            nc.scalar.copy(wT[:], wT_ps[:])

        # ---- residents ----
        xc = resident.tile([C, N], F16)
        xt = resident.tile([128, N], F16)
        a_t = resident.tile([128, N // 16], F16)

        # ================= phase 1 =================
        att_pool = ctx.enter_context(tc.tile_pool(name="att_ps", bufs=2, space="PSUM"))
        xt_pool = ctx.enter_context(tc.tile_pool(name="xt_ps", bufs=2, space="PSUM"))
        pq_pool = ctx.enter_context(tc.tile_pool(name="pq_ps", bufs=1, space="PSUM"))
        att_hist = []   # last two groups' att psum tiles (full banks)
        xtp_hist = []   # last two groups' xtp psum tile pairs
        if True:
            pq = pq_pool.tile([G, 2 * C], F32)   # [P^T | Q^T]

            def emit_pq(gg, x2tile):
                """P/Q matmuls for group gg (deferred one group)."""
                for j in range(JJ):
                    t = gg * JJ + j
                    at8 = a_t[:, 8 * t:8 * (t + 1)]
                    nc.tensor.matmul(pq[:, 0:C], lhsT=at8,
                                     rhs=xt[:, t * TP:(t + 1) * TP],
                                     start=(t == 0), stop=(t == ntiles - 1),
                                     skip_group_check=True)
                    nc.tensor.matmul(pq[:, C:2 * C], lhsT=at8,
                                     rhs=x2tile[:, j * TP:(j + 1) * TP],
                                     start=False, stop=(t == ntiles - 1),
                                     skip_group_check=True)

            prev = None  # (group idx, x2 tile)
            dmajunk = stats.tile([1, 4], F16)
            for g in range(ngrp):
                n0 = g * GRP
                xs = xstage.tile([C, GRP], F32)
                if g >= 2:
                    # SP-queue clock sync: observe ACT past xc-copy(g-2) so the
                    # bulk DMA below only needs its DMAHW (slot WAW) wait.
                    nc.sync.dma_start(dmajunk[:], xc[0:1, (g - 2) * GRP:(g - 2) * GRP + 4])
                nc.sync.dma_start(xs[:], x_ext[:, n0:n0 + GRP])
                xcs = xc[:, n0:n0 + GRP]
                nc.scalar.copy(xcs, xs[:])

                att_bank = att_pool.tile([128, 512], F32)
                att_ps = att_bank[:, 0:JJ * G].rearrange("p (j g) -> p j g", g=G)
                xtp = [xt_pool.tile([128, 8 * TP], F16, name=f"xtp{h}", tag=f"xtp{h}")
                       for h in range(2)]
                att_hist.append(att_bank)
                xtp_hist.append(xtp)
                att_hist = att_hist[-2:]
                xtp_hist = xtp_hist[-2:]
                for j in range(JJ):
                    xcj = xc[:, n0 + j * TP:n0 + (j + 1) * TP]
                    nc.tensor.matmul(
                        xtp[j // 8][:, (j % 8) * TP:(j % 8 + 1) * TP], lhsT=xcj,
                        rhs=ident[:], is_transpose=True,
                        start=(j % 8 == 0), stop=(j % 8 == 7), skip_group_check=True)
                    nc.tensor.matmul(
                        att_ps[:, j, :], lhsT=xcj, rhs=wT[:],
                        start=(j == 0), stop=(j == JJ - 1) and not fold_bias,
                        skip_group_check=True)
                if fold_bias:
                    nc.tensor.matmul(
                        att_bank[:, 0:JJ * G],
                        lhsT=ones_col[:], rhs=b_rep[:].rearrange("p j g -> p (j g)"),
                        start=False, stop=True, skip_group_check=True)

                # previous group's P/Q now that this group's ACT wait is in place
                if prev is not None:
                    emit_pq(*prev)
                    # tiny ACT read of the previous group's last DVE output:
                    # advances ACT's observed DVE clock so exp below doesn't
                    # need a third (DVE WAR) wait slot.
                    sync_junk = stats.tile([128, 1], F16, tag="sync_junk")
                    nc.scalar.copy(sync_junk[:], prev[1][:, 0:1])

                e = estage.tile([128, JJ, G], F32)
                nc.scalar.activation(e[:], att_ps, AF.Exp)
                z = zstage.tile([128, JJ], F32)
                nc.vector.tensor_reduce(z[:], e[:], axis=AX.X, op=OP.add)
                rz = zstage.tile([128, JJ], F32)
                nc.vector.reciprocal(rz[:], z[:])
                atG = a_t[:, g * (GRP // 16):(g + 1) * (GRP // 16)].rearrange(
                    "p (j g) -> p j g", g=G)
                nc.vector.tensor_tensor(atG, e[:], bcast_last(rz[:], G), op=OP.mult)

                for h in range(2):
                    nc.scalar.copy(xt[:, n0 + h * 8 * TP:n0 + (h + 1) * 8 * TP],
                                   xtp[h][:])
                x2s = x2stage.tile([128, GRP], F16)
                nc.vector.tensor_tensor(x2s[:], xt[:, n0:n0 + GRP],
                                        xt[:, n0:n0 + GRP], op=OP.mult)
                prev = (g, x2s)

            emit_pq(*prev)

            # ================= phase 1.5: statistics =================
            partials = stats.tile([128, G], F32)
            nc.vector.tensor_reduce(
                partials[:], a_t[:].rearrange("p (t g) -> p g t", g=G),
                axis=AX.X, op=OP.add)
            with tc.tile_pool(name="s_ps", bufs=1, space="PSUM") as s_pool:
                s_ps = s_pool.tile([G, 1], F32)
                nc.tensor.matmul(s_ps[:], lhsT=partials[:], rhs=onesf32[:],
                                 start=True, stop=True)
                s_eps = stats.tile([G, 1], F32)
                nc.vector.tensor_scalar_add(s_eps[:], s_ps[:], A_NORM_EPS)
                sden = stats.tile([G, 1], F32)
                nc.vector.reciprocal(sden[:], s_eps[:])
                T = stats.tile([G, 1], F32)
                nc.vector.tensor_tensor(T[:], s_ps[:], sden[:], op=OP.mult)
                meanT = stats.tile([G, C], F32)
                nc.vector.tensor_scalar_mul(meanT[:], pq[:, 0:C], sden[:])
                m2T = stats.tile([G, C], F32)
                nc.vector.tensor_scalar_mul(m2T[:], pq[:, C:2 * C], sden[:])
            u = stats.tile([G, 1], F32)
            nc.vector.tensor_scalar(u[:], T[:], -1.0, 2.0, op0=OP.mult, op1=OP.add)
            meansq = stats.tile([G, C], F32)
            nc.vector.tensor_tensor(meansq[:], meanT[:], meanT[:], op=OP.mult)
            tmpv = stats.tile([G, C], F32)
            nc.vector.tensor_scalar_mul(tmpv[:], meansq[:], u[:])
            varT = stats.tile([G, C], F32)
            nc.vector.tensor_tensor(varT[:], m2T[:], tmpv[:], op=OP.subtract)
            lnv = stats.tile([G, C], F32)
            nc.scalar.activation(lnv[:], varT[:], AF.Ln, bias=eps_t[:])
            invT = stats.tile([G, C], F32)
            nc.scalar.activation(invT[:], lnv[:], AF.Exp, scale=-0.5)
            Ff = stats.tile([G, C], F32)
            nc.vector.tensor_tensor(Ff[:], meanT[:], invT[:], op=OP.mult)
            # E/F land on ACT so A-matmuls wait on ACT alone
            E = stats.tile([G, C], F16)
            nc.scalar.copy(E[:], invT[:])
            F = stats.tile([G, C], F16)
            nc.scalar.copy(F[:], Ff[:])

        # ================= phase 2: apply =================
        # PSUM: re-write the still-live phase-1 tiles (same-tile WAW on the
        # same engine is free). SBUF: explicit double-buffered tiles for the
        # same reason - pool slot-recycling would emit own-engine release
        # waits that blow the per-instruction wait budget.
        NCH = 1024
        HC = 512
        if len(att_hist) == 1:
            att_hist = [att_hist[0], att_hist[0]]
            xtp_hist = [xtp_hist[0], xtp_hist[0]]
        ags_db = [ag_sbp.tile([G, NCH], F16, name=f"ags{i}", tag=f"ags{i}")
                  for i in range(2)]
        a1s_db = [a12_sbp.tile([C, HC], F16, name=f"a1s{i}", tag=f"a1s{i}")
                  for i in range(2)]
        a2s_db = [a12_sbp.tile([C, HC], F16, name=f"a2s{i}", tag=f"a2s{i}")
                  for i in range(2)]
        tm_db = [tmp_pool.tile([C, HC], F16, name=f"tm{i}", tag=f"tm{i}")
                 for i in range(2)]
        os_db = [opool.tile([C, NCH], F16, name=f"os{i}", tag=f"os{i}")
                 for i in range(2)]
        junk3 = stats.tile([1, 1], F16)
        junk4 = stats.tile([1, 1], F16)
        for cc in range(N // NCH):
            n0 = cc * NCH
            att_bank = att_hist[cc % 2]
            agp = att_bank[0:8, :].bitcast(F16).rearrange("p (r t) -> p r t", t=TP)
            for r in range(NCH // TP):
                t = cc * (NCH // TP) + r
                nc.tensor.matmul(agp[:, r, :], lhsT=a_t[:, 8 * t:8 * (t + 1)],
                                 rhs=ident[:], is_transpose=True,
                                 start=(r == 0), stop=(r == NCH // TP - 1),
                                 skip_group_check=True)
            ags = ags_db[cc % 2]
            nc.scalar.copy(ags[:], agp.rearrange("p r t -> p (r t)"))
            if cc >= 1:
                # clock-syncs: ACT observes gpsimd (pass1), gpsimd observes
                # DVE (pass2), so the copies below stay within wait budget.
                nc.scalar.copy(junk3[:], tm_db[1][0:1, 0:1])
                nc.gpsimd.tensor_copy(junk4[:], os_db[(cc - 1) % 2][0:1, NCH - 1:NCH])
            os = os_db[cc % 2]
            for h in range(2):
                m0 = h * HC
                xpair = xtp_hist[h]
                a1p = xpair[0][:].bitcast(F32)   # [C, 512] psum view
                a2p = xpair[1][:].bitcast(F32)
                nc.tensor.matmul(a1p, lhsT=E[:], rhs=ags[:, m0:m0 + HC],
                                 start=True, stop=True, skip_group_check=True)
                nc.tensor.matmul(a2p, lhsT=F[:], rhs=ags[:, m0:m0 + HC],
                                 start=True, stop=True, skip_group_check=True)
                a1s = a1s_db[h]
                nc.scalar.copy(a1s[:], a1p)
                a2s = a2s_db[h]
                nc.vector.tensor_copy(a2s[:], a2p)
                tm = tm_db[h]
                nc.gpsimd.tensor_tensor(tm[:], xc[:, n0 + m0:n0 + m0 + HC],
                                        a1s[:], op=OP.mult)
                nc.vector.tensor_tensor(os[:, m0:m0 + HC], tm[:], a2s[:],
                                        op=OP.subtract)
            nc.sync.dma_start(out_ext[:, n0:n0 + NCH], os[:])

    nc.compile()
    return nc


_CACHED_NC = None


def kernel(x, conv_w, conv_b):
    global _CACHED_NC
    x = np.asarray(x)
    conv_w = np.ascontiguousarray(conv_w, dtype=np.float32)
    conv_b = np.asarray(conv_b, dtype=np.float32)
    b, c, n = x.shape[0], x.shape[1], x.shape[2]
    if _CACHED_NC is None:
        _CACHED_NC = build_nc(N=n)
    nc = _CACHED_NC

    in_maps = [
        {
            "x": np.ascontiguousarray(x[i, :, :, 0], dtype=np.float32),
            "conv_w": conv_w,
            "conv_b": conv_b.reshape(1, -1),
        }
        for i in range(b)
    ]
    res = run_bass_kernel_spmd(nc, in_maps, core_ids=list(range(b)))
    out = np.stack([res.results[i]["out"] for i in range(b)])[..., None]
    return out.astype(np.float32)



# revision 24
# speedup vs baseline: 1.4955x; 1.2162x over previous
"""ACN2d multi-branch attentive normalization on 8 TRN2 NeuronCores.

Sharding: data-parallel over batch B (8 samples -> 8 cores), no collectives.
Per core, a two-phase Bass/Tile kernel:
  phase 1 (point-major): PE transposes x to fp16 point-major tiles and
    computes att^T; softmax over the G=8 groups is a free-dim reduce on DVE;
    weighted moments P = x a^T, Q = x^2 a^T and s = sum_n a accumulate on PE.
  phase 2 (channel-major): a^T transposes back to group-major chunks on PE;
    A1/A2 are K=8 matmuls against inv_std^T / (mean inv_std)^T; gpsimd/DVE
    combine out = x*A1 - A2 and results stream to HBM.
"""

import numpy as np

from concourse.bass_utils import run_bass_kernel_spmd


from contextlib import ExitStack

import concourse.bass as bass
import concourse.bacc as bacc
import concourse.tile as tile
from concourse import mybir
from concourse.masks import make_identity

F32 = mybir.dt.float32
F16 = mybir.dt.float16
AF = mybir.ActivationFunctionType
OP = mybir.AluOpType
AX = mybir.AxisListType

EPS = 1e-3
A_NORM_EPS = 1e-8


def bcast_last(ap, n):
    return bass.AP(tensor=ap.tensor, offset=ap.offset, ap=list(ap.ap) + [[0, n]])


def build_nc(N=32768, fold_bias=True):
    C, G = 128, 8
    TP = 128
    GRP = 2048
    JJ = GRP // TP                # tiles per group = 16
    ntiles = N // TP
    ngrp = N // GRP
    assert N % GRP == 0

    nc = bacc.Bacc("TRN2", target_bir_lowering=False, debug=False)
    x_ext = nc.declare_dram_parameter("x", [C, N], F32, isOutput=False).ap()
    w_ext = nc.declare_dram_parameter("conv_w", [G, C], F32, isOutput=False).ap()
    b_ext = nc.declare_dram_parameter("conv_b", [1, G], F32, isOutput=False).ap()
    out_ext = nc.declare_dram_parameter("out", [C, N], F16, isOutput=True).ap()

    with tile.TileContext(nc) as tc, ExitStack() as ctx:
        consts = ctx.enter_context(tc.tile_pool(name="consts", bufs=1))
        resident = ctx.enter_context(tc.tile_pool(name="resident", bufs=1))
        stats = ctx.enter_context(tc.tile_pool(name="stats", bufs=1))
        # SBUF staging pools stay open for the whole kernel so later pools
        # never inherit WAW deps on the DMA lanes that filled earlier ones.
        xstage = ctx.enter_context(tc.tile_pool(name="xstage", bufs=2))
        x2stage = ctx.enter_context(tc.tile_pool(name="x2stage", bufs=2))
        estage = ctx.enter_context(tc.tile_pool(name="estage", bufs=2))
        zstage = ctx.enter_context(tc.tile_pool(name="zstage", bufs=2))
        ag_sbp = ctx.enter_context(tc.tile_pool(name="ag_sb", bufs=2))
        a12_sbp = ctx.enter_context(tc.tile_pool(name="a12_sb", bufs=2))
        tmp_pool = ctx.enter_context(tc.tile_pool(name="tmp_sb", bufs=2))
        opool = ctx.enter_context(tc.tile_pool(name="ostage", bufs=2))

        # ---- constants: everything the PE touches funnels through gpsimd ----
        ident = consts.tile([128, 128], F16)
        make_identity(nc, ident)
        ident8 = consts.tile([G, G], F32)
        make_identity(nc, ident8)
        onesf32 = consts.tile([128, 1], F32)
        nc.vector.memset(onesf32, 1.0)
        eps_t = consts.tile([G, 1], F32)
        nc.vector.memset(eps_t, EPS)
        w_sb = consts.tile([G, C], F32)
        nc.sync.dma_start(w_sb[:], w_ext)
        b_row = consts.tile([1, G], F32)
        nc.sync.dma_start(b_row[:], b_ext)
        ones_col = consts.tile([1, 128], F16)
        nc.gpsimd.memset(ones_col, 1.0)
        b_rep = consts.tile([1, JJ, G], F16)
        nc.gpsimd.tensor_copy(
            b_rep[:],
            bass.AP(tensor=b_row[:].tensor, offset=b_row[:].offset,
                    ap=[b_row[:].ap[0], [0, JJ], b_row[:].ap[1]]))
        # bounce conv_w through gpsimd so the wT transpose waits on Pool only
        w_g = consts.tile([G, C], F32)
        nc.gpsimd.tensor_copy(w_g[:], w_sb[:])

        with tc.tile_pool(name="ph0psum", bufs=1, space="PSUM") as ph0psum:
            wT_ps = ph0psum.tile([C, G], F32)
            nc.tensor.transpose(wT_ps[:], w_g[:], ident8[:])
            wT = consts.tile([C, G], F16)
            nc.scalar.copy(wT[:], wT_ps[:])

        # ---- residents ----
        xc = resident.tile([C, N], F16)
        xt = resident.tile([128, N], F16)
        a_t = resident.tile([128, N // 16], F16)

        # ================= phase 1 =================
        att_pool = ctx.enter_context(tc.tile_pool(name="att_ps", bufs=2, space="PSUM"))
        xt_pool = ctx.enter_context(tc.tile_pool(name="xt_ps", bufs=2, space="PSUM"))
        pq_pool = ctx.enter_context(tc.tile_pool(name="pq_ps", bufs=1, space="PSUM"))
        att_hist = []   # last two groups' att psum tiles (full banks)
        xtp_hist = []   # last two groups' xtp psum tile pairs
        if True:
            pq = pq_pool.tile([G, 2 * C], F32)   # [P^T | Q^T]

            def emit_pq(gg, x2tile):
                """P/Q matmuls for group gg (deferred one group)."""
                for j in range(JJ):
                    t = gg * JJ + j
                    at8 = a_t[:, 8 * t:8 * (t + 1)]
                    nc.tensor.matmul(pq[:, 0:C], lhsT=at8,
                                     rhs=xt[:, t * TP:(t + 1) * TP],
                                     start=(t == 0), stop=(t == ntiles - 1),
                                     skip_group_check=True)
                    nc.tensor.matmul(pq[:, C:2 * C], lhsT=at8,
                                     rhs=x2tile[:, j * TP:(j + 1) * TP],
                                     start=False, stop=(t == ntiles - 1),
                                     skip_group_check=True)

            prev = None  # (group idx, x2 tile)
            dmajunk = stats.tile([1, 4], F16)
            for g in range(ngrp):
                n0 = g * GRP
                xs = xstage.tile([C, GRP], F32)
                if g >= 2:
                    # SP-queue clock sync: observe ACT past xc-copy(g-2) so the
                    # bulk DMA below only needs its DMAHW (slot WAW) wait.
                    nc.sync.dma_start(dmajunk[:], xc[0:1, (g - 2) * GRP:(g - 2) * GRP + 4])
                nc.sync.dma_start(xs[:], x_ext[:, n0:n0 + GRP])
                xcs = xc[:, n0:n0 + GRP]
                nc.vector.tensor_copy(xcs, xs[:])

                att_bank = att_pool.tile([128, 512], F32)
                att_ps = att_bank[:, 0:JJ * G].rearrange("p (j g) -> p j g", g=G)
                xtp = [xt_pool.tile([128, 8 * TP], F16, name=f"xtp{h}", tag=f"xtp{h}")
                       for h in range(2)]
                att_hist.append(att_bank)
                xtp_hist.append(xtp)
                att_hist = att_hist[-2:]
                xtp_hist = xtp_hist[-2:]
                for j in range(JJ):
                    xcj = xc[:, n0 + j * TP:n0 + (j + 1) * TP]
                    nc.tensor.matmul(
                        xtp[j // 8][:, (j % 8) * TP:(j % 8 + 1) * TP], lhsT=xcj,
                        rhs=ident[:], is_transpose=True,
                        start=(j % 8 == 0), stop=(j % 8 == 7), skip_group_check=True)
                    nc.tensor.matmul(
                        att_ps[:, j, :], lhsT=xcj, rhs=wT[:],
                        start=(j == 0), stop=(j == JJ - 1) and not fold_bias,
                        skip_group_check=True)
                if fold_bias:
                    nc.tensor.matmul(
                        att_bank[:, 0:JJ * G],
                        lhsT=ones_col[:], rhs=b_rep[:].rearrange("p j g -> p (j g)"),
                        start=False, stop=True, skip_group_check=True)

                # previous group's P/Q now that this group's ACT wait is in place
                if prev is not None:
                    emit_pq(*prev)
                    # tiny ACT read of the previous group's last DVE output:
                    # advances ACT's observed DVE clock so exp below doesn't
                    # need a third (DVE WAR) wait slot.
                    sync_junk = stats.tile([128, 1], F16, tag="sync_junk")
                    nc.scalar.copy(sync_junk[:], prev[1][:, 0:1])

                e = estage.tile([128, JJ, G], F32)
                nc.scalar.activation(e[:], att_ps, AF.Exp)
                z = zstage.tile([128, JJ], F32)
                nc.vector.tensor_reduce(z[:], e[:], axis=AX.X, op=OP.add)
                rz = zstage.tile([128, JJ], F32)
                nc.vector.reciprocal(rz[:], z[:])
                atG = a_t[:, g * (GRP // 16):(g + 1) * (GRP // 16)].rearrange(
                    "p (j g) -> p j g", g=G)
                nc.vector.tensor_tensor(atG, e[:], bcast_last(rz[:], G), op=OP.mult)

                for h in range(2):
                    nc.scalar.copy(xt[:, n0 + h * 8 * TP:n0 + (h + 1) * 8 * TP],
                                   xtp[h][:])
                x2s = x2stage.tile([128, GRP], F16)
                nc.vector.tensor_tensor(x2s[:], xt[:, n0:n0 + GRP],
                                        xt[:, n0:n0 + GRP], op=OP.mult)
                prev = (g, x2s)

            emit_pq(*prev)

            # ================= phase 1.5: statistics =================
            partials = stats.tile([128, G], F32)
            nc.vector.tensor_reduce(
                partials[:], a_t[:].rearrange("p (t g) -> p g t", g=G),
                axis=AX.X, op=OP.add)
            with tc.tile_pool(name="s_ps", bufs=1, space="PSUM") as s_pool:
                s_ps = s_pool.tile([G, 1], F32)
                nc.tensor.matmul(s_ps[:], lhsT=partials[:], rhs=onesf32[:],
                                 start=True, stop=True)
                s_eps = stats.tile([G, 1], F32)
                nc.vector.tensor_scalar_add(s_eps[:], s_ps[:], A_NORM_EPS)
                sden = stats.tile([G, 1], F32)
                nc.vector.reciprocal(sden[:], s_eps[:])
                T = stats.tile([G, 1], F32)
                nc.vector.tensor_tensor(T[:], s_ps[:], sden[:], op=OP.mult)
                meanT = stats.tile([G, C], F32)
                nc.vector.tensor_scalar_mul(meanT[:], pq[:, 0:C], sden[:])
                m2T = stats.tile([G, C], F32)
                nc.vector.tensor_scalar_mul(m2T[:], pq[:, C:2 * C], sden[:])
            u = stats.tile([G, 1], F32)
            nc.vector.tensor_scalar(u[:], T[:], -1.0, 2.0, op0=OP.mult, op1=OP.add)
            meansq = stats.tile([G, C], F32)
            nc.vector.tensor_tensor(meansq[:], meanT[:], meanT[:], op=OP.mult)
            tmpv = stats.tile([G, C], F32)
            nc.vector.tensor_scalar_mul(tmpv[:], meansq[:], u[:])
            varT = stats.tile([G, C], F32)
            nc.vector.tensor_tensor(varT[:], m2T[:], tmpv[:], op=OP.subtract)
            lnv = stats.tile([G, C], F32)
            nc.scalar.activation(lnv[:], varT[:], AF.Ln, bias=eps_t[:])
            invT = stats.tile([G, C], F32)
            nc.scalar.activation(invT[:], lnv[:], AF.Exp, scale=-0.5)
            Ff = stats.tile([G, C], F32)
            nc.vector.tensor_tensor(Ff[:], meanT[:], invT[:], op=OP.mult)
            # E/F land on ACT so A-matmuls wait on ACT alone
            E = stats.tile([G, C], F16)
            nc.scalar.copy(E[:], invT[:])
            F = stats.tile([G, C], F16)
            nc.scalar.copy(F[:], Ff[:])

        # ================= phase 2: apply =================
        # PSUM: re-write the still-live phase-1 tiles (same-tile WAW on the
        # same engine is free). SBUF: explicit double-buffered tiles for the
        # same reason - pool slot-recycling would emit own-engine release
        # waits that blow the per-instruction wait budget.
        NCH = 1024
        HC = 512
        if len(att_hist) == 1:
            att_hist = [att_hist[0], att_hist[0]]
            xtp_hist = [xtp_hist[0], xtp_hist[0]]
        ags_db = [ag_sbp.tile([G, NCH], F16, name=f"ags{i}", tag=f"ags{i}")
                  for i in range(2)]
        a1s_db = [a12_sbp.tile([C, HC], F16, name=f"a1s{i}", tag=f"a1s{i}")
                  for i in range(2)]
        a2s_db = [a12_sbp.tile([C, HC], F16, name=f"a2s{i}", tag=f"a2s{i}")
                  for i in range(2)]
        tm_db = [tmp_pool.tile([C, HC], F16, name=f"tm{i}", tag=f"tm{i}")
                 for i in range(2)]
        os_db = [opool.tile([C, NCH], F16, name=f"os{i}", tag=f"os{i}")
                 for i in range(2)]
        junk3 = stats.tile([1, 1], F16)
        junk4 = stats.tile([1, 1], F16)
        for cc in range(N // NCH):
            n0 = cc * NCH
            att_bank = att_hist[cc % 2]
            agp = att_bank[0:8, :].bitcast(F16).rearrange("p (r t) -> p r t", t=TP)
            for r in range(NCH // TP):
                t = cc * (NCH // TP) + r
                nc.tensor.matmul(agp[:, r, :], lhsT=a_t[:, 8 * t:8 * (t + 1)],
                                 rhs=ident[:], is_transpose=True,
                                 start=(r == 0), stop=(r == NCH // TP - 1),
                                 skip_group_check=True)
            ags = ags_db[cc % 2]
            agsf = agp.rearrange("p r t -> p (r t)")
            nc.scalar.copy(ags[:, 0:NCH // 2], agsf[:, 0:NCH // 2])
            nc.vector.tensor_copy(ags[:, NCH // 2:NCH], agsf[:, NCH // 2:NCH])
            if cc >= 1:
                # clock-syncs: ACT observes gpsimd (pass1), gpsimd observes
                # DVE (pass2), so the copies below stay within wait budget.
                nc.scalar.copy(junk3[:], tm_db[1][0:1, 0:1])
                nc.gpsimd.tensor_copy(junk4[:], os_db[(cc - 1) % 2][0:1, NCH - 1:NCH])
            os = os_db[cc % 2]
            for h in range(2):
                m0 = h * HC
                xpair = xtp_hist[h]
                a1p = xpair[0][:].bitcast(F32)   # [C, 512] psum view
                a2p = xpair[1][:].bitcast(F32)
                nc.tensor.matmul(a1p, lhsT=E[:], rhs=ags[:, m0:m0 + HC],
                                 start=True, stop=True, skip_group_check=True)
                nc.tensor.matmul(a2p, lhsT=F[:], rhs=ags[:, m0:m0 + HC],
                                 start=True, stop=True, skip_group_check=True)
                a1s = a1s_db[h]
                nc.scalar.copy(a1s[:], a1p)
                tm = tm_db[h]
                nc.gpsimd.tensor_tensor(tm[:], xc[:, n0 + m0:n0 + m0 + HC],
                                        a1s[:], op=OP.mult)
                nc.vector.tensor_tensor(os[:, m0:m0 + HC], tm[:], a2p,
                                        op=OP.subtract)
            nc.sync.dma_start(out_ext[:, n0:n0 + NCH], os[:])

    nc.compile()
    return nc


_CACHED_NC = None


def kernel(x, conv_w, conv_b):
    global _CACHED_NC
    x = np.asarray(x)
    conv_w = np.ascontiguousarray(conv_w, dtype=np.float32)
    conv_b = np.asarray(conv_b, dtype=np.float32)
    b, c, n = x.shape[0], x.shape[1], x.shape[2]
    if _CACHED_NC is None:
        _CACHED_NC = build_nc(N=n)
    nc = _CACHED_NC

    in_maps = [
        {
            "x": np.ascontiguousarray(x[i, :, :, 0], dtype=np.float32),
            "conv_w": conv_w,
            "conv_b": conv_b.reshape(1, -1),
        }
        for i in range(b)
    ]
    res = run_bass_kernel_spmd(nc, in_maps, core_ids=list(range(b)))
    out = np.stack([res.results[i]["out"] for i in range(b)])[..., None]
    return out.astype(np.float32)

